# revision 1
# baseline (speedup 1.0000x reference)
"""Trainium2 Bass kernel for nn_ExternalEmbeddingAttention.

Sharding: data-parallel over batch B=8 across 8 NeuronCores (one example per
core); weights replicated.

Host constant-folds (weight-only algebra, computed once in fp64 in kernel()):
  W*   = Wq @ Wk.T        W*T = Wk @ Wq.T        Wvo = Wv @ Wo
  Wcat = [W* | Wvo]   ([H, 2H], shares one stationary per token/k tile)
Per-core device algorithm (token-major layout; per-token scalars live on
partitions so the softmax folds become tensor_scalar ops):
  ext MLP + LN on device (W1/W2 streamed in chunks) -> extLN
  A    = extLN @ W*T.T   (== Wq @ k_ext.T), then transposed to [H, E]
  wv'  = gamma * (extLN @ Wvo)   (== gamma * (v_ext @ Wo))
  per token tile: [u | ov] = hs @ Wcat ; s_ext = hs @ A
  s_self = rowsum(u * hs); softmax normalization folded past Wo:
    out_attn = (e0*rZ)*ov + (eext*rZ) @ wv'
  out = LN(out_attn + hs); rstd = Exp(-0.5*Ln(var+eps)) (single ACT table).
All matmuls run in float32r (TF32-grade, 1 cyc/row at N>=256).
"""

import numpy as np

import concourse.bass as bass
import concourse.tile as tile
import concourse.mybir as mybir
from concourse import bacc
from concourse.bass_utils import run_bass_kernel_spmd
from concourse.masks import make_identity
import concourse.bass_utils as _bass_utils

# Walrus's LDWEIGHTS-dedup pass is disabled by default in this harness; with
# fused f32r matmuls every MATMUL re-loads its stationary operand, which
# costs ~50us of PE time here. Re-enable it for our compiles only.
_orig_run_command = _bass_utils.run_command


def _run_command_ldwopt(argv, **kwargs):
    argv = ["--enable-ldw-opt=true" if a == "--enable-ldw-opt=false" else a
            for a in argv]
    return _orig_run_command(argv, **kwargs)


if _bass_utils.run_command is not _run_command_ldwopt:
    _bass_utils.run_command = _run_command_ldwopt

# Steer the act-table chooser: Exp and Ln both live in
# natural_log_exp_and_others, but the chooser's first-match picks sets that
# hold only one of them, reloading tables (~1.3us each) every iteration.
# Restrict Exp/Ln to the shared set (a pure choice restriction - that set
# genuinely contains both, so results are unchanged).
from concourse.hw_specs import get_activation_tables as _gat


def _steer_act_tables(arch="gen3"):
    t = _gat(arch)   # functools.cache -> in-place mutation persists
    for name, funcs in t.items():
        if name != "natural_log_exp_and_others":
            funcs.discard(mybir.ActivationFunctionType.Exp)
            funcs.discard(mybir.ActivationFunctionType.Ln)


_steer_act_tables()

F32 = mybir.dt.float32
F32R = mybir.dt.float32r
AF = mybir.ActivationFunctionType
OP = mybir.AluOpType

B, S, H, E, I = 8, 2048, 768, 16, 3072
EPS = 1e-12
P = 128
KO = H // P          # 6 k-tiles over a 768 dim
TT = S // P          # 16 token tiles
H2 = 2 * H


def _tp(nc, psum_pool, src_ap, dst_ap, ident, eng="any"):
    """PE-transpose src [p, f] -> dst [f, p] via PSUM (f32r)."""
    pdim = src_ap.shape[-1]
    fdim = src_ap.shape[0]
    ps = psum_pool.tile([128, 128], F32R, tag="tp")
    nc.tensor.transpose(ps[:pdim, :fdim], src_ap, ident[:fdim, :fdim])
    if eng == "act":
        nc.scalar.copy(dst_ap, ps[:pdim, :fdim])
    elif eng == "dve":
        nc.vector.tensor_copy(dst_ap, ps[:pdim, :fdim])
    else:
        nc.any.tensor_copy(dst_ap, ps[:pdim, :fdim])


def _build(use_bias: dict, dbg: bool = False):
    nc = bacc.Bacc()

    hs_d = nc.dram_tensor("hs", [S, H], F32R, kind="ExternalInput")
    ext_d = nc.dram_tensor("ext", [E, H], F32R, kind="ExternalInput")
    dl_d = nc.dram_tensor("dl", [E, 1], F32, kind="ExternalInput")
    wcat_d = nc.dram_tensor("Wcat", [H, H2], F32R, kind="ExternalInput")
    wstarT_d = nc.dram_tensor("WstarT", [H, H], F32R, kind="ExternalInput")
    wvo_d = nc.dram_tensor("Wvo", [H, H], F32R, kind="ExternalInput")
    w1_d = nc.dram_tensor("W1", [H, I], F32R, kind="ExternalInput")
    w2_d = nc.dram_tensor("W2", [I, H], F32R, kind="ExternalInput")
    bias_d = {}
    for nm, sz in (("b1", I), ("b2", H), ("bo", H), ("mlp_g", H),
                   ("mlp_b", H), ("ln_g", H), ("ln_b", H), ("dvec", H),
                   ("c0", 1), ("wkbq", H), ("bqbk", 1), ("bvwo", H)):
        if use_bias.get(nm):
            bias_d[nm] = nc.dram_tensor(nm, [1, sz], F32, kind="ExternalInput")
    out_d = nc.dram_tensor("out", [S, H], F32, kind="ExternalOutput")
    dbg_d = {}
    if dbg:
        for nm, shp in (("d_extLN", [E, H]), ("d_A", [E, H]),
                        ("d_ss", [P, TT]), ("d_sext", [S, E])):
            dbg_d[nm] = nc.dram_tensor(nm, shp, F32, kind="ExternalOutput")

    with tile.TileContext(nc) as tc:
        with tc.tile_pool(name="persist", bufs=1) as persist:
            ident_f = persist.tile([128, 128], F32, tag="ident_f")
            make_identity(nc, ident_f)
            ident = persist.tile([128, 128], F32R, tag="ident")
            nc.vector.tensor_copy(ident, ident_f)
            eps_t = persist.tile([128, 1], F32, tag="eps")
            nc.vector.memset(eps_t, EPS)
            dl_t = persist.tile([E, 1], F32, tag="dl")
            nc.sync.dma_start(dl_t, dl_d[:])

            bias_t = {}
            for nm, d in bias_d.items():
                sz = d.shape[1]
                pp = E if nm in ("b1", "b2", "mlp_g", "mlp_b", "wkbq") else P
                t = persist.tile([pp, sz], F32, tag=f"bias_{nm}",
                                 name=f"bias_{nm}")
                nc.gpsimd.dma_start(t, d[:].to_broadcast((pp, sz)))
                bias_t[nm] = t

            # persistent P-phase products
            a_t = persist.tile([128, KO, E], F32R, tag="a_t")
            wvext = persist.tile([E, H], F32R, tag="wvext")
            extT = persist.tile([128, KO, E], F32R, tag="extT")
            extLN = persist.tile([E, H], F32R, tag="extLN")
            extLNT = persist.tile([128, KO, E], F32R, tag="extLNT")
            cvec_bc = (persist.tile([128, E], F32, tag="cvec_bc")
                       if use_bias.get("wkbq") else None)

            # big persistent data (loaded early; hs per-tile for pipelining)
            hs_sb = persist.tile([128, TT, H], F32R, tag="hs")
            hs_r = hs_d.rearrange("(tt p) h -> p tt h", p=128)
            for tt in range(TT):
                nc.sync.dma_start(hs_sb[:, tt], hs_r[:, tt])
            wcat_sb = persist.tile([128, KO, H2], F32R, tag="wcat")
            nc.sync.dma_start(wcat_sb,
                              wcat_d.rearrange("(ko p) n -> p ko n", p=128))
            hsT = persist.tile([128, KO, S], F32R, tag="hsT")

            # ---------------- P1: ext MLP ----------------
            ext_t = persist.tile([E, H], F32R, tag="ext_t")
            nc.sync.dma_start(ext_t, ext_d[:])
            with tc.tile_pool(name="p1_tp", bufs=2, space="PSUM") as p1_tp:
                for k in range(KO):
                    _tp(nc, p1_tp, ext_t[:, k * P:(k + 1) * P], extT[:, k],
                        ident)

                with tc.tile_pool(name="wstream", bufs=2) as wstream, \
                     tc.tile_pool(name="mlp_h1", bufs=2, space="PSUM") as mh1, \
                     tc.tile_pool(name="mlp_h2", bufs=1, space="PSUM") as mh2, \
                     tc.tile_pool(name="mlp_sb", bufs=2) as mlp_sb:
                    h1gT = mlp_sb.tile([128, I // P, E], F32R, tag="h1gT")
                    h2_ps = mh2.tile([E, H], F32, tag="h2")
                    for c in range(I // 512):
                        w1c = wstream.tile([128, KO, 512], F32R, tag="w1c")
                        nc.sync.dma_start(
                            w1c, w1_d.rearrange("(ko p) n -> p ko n", p=128)
                            [:, :, c * 512:(c + 1) * 512])
                        h1_ps = mh1.tile([E, 512], F32, tag="h1")
                        for k in range(KO):
                            nc.tensor.matmul(h1_ps, extT[:, k], w1c[:, k],
                                             start=(k == 0),
                                             stop=(k == KO - 1))
                        if use_bias.get("b1"):
                            nc.vector.tensor_add(
                                h1_ps, h1_ps,
                                bias_t["b1"][:E, c * 512:(c + 1) * 512])
                        h1g = mlp_sb.tile([E, 512], F32R, tag="h1g")
                        nc.scalar.activation(h1g, h1_ps, AF.Gelu)
                        for j in range(4):
                            _tp(nc, p1_tp, h1g[:, j * P:(j + 1) * P],
                                h1gT[:, c * 4 + j], ident)
                        w2c = wstream.tile([128, 4, H], F32R, tag="w2c")
                        nc.sync.dma_start(
                            w2c, w2_d.rearrange("(jo p) n -> p jo n", p=128)
                            [:, c * 4:(c + 1) * 4, :])
                        for j in range(4):
                            for off, ln in ((0, 512), (512, 256)):
                                nc.tensor.matmul(
                                    h2_ps[:, off:off + ln],
                                    h1gT[:, c * 4 + j],
                                    w2c[:, j, off:off + ln],
                                    start=(c == 0 and j == 0),
                                    stop=(c == I // 512 - 1 and j == 3))
                    # residual + LN over free dim (16 partitions)
                    z = mlp_sb.tile([E, H], F32, tag="z")
                    nc.vector.tensor_add(z, h2_ps, ext_t.bitcast(F32))
                    if use_bias.get("b2"):
                        nc.vector.tensor_add(z, z, bias_t["b2"][:E])
                    stats = mlp_sb.tile([E, 3, 6], F32, tag="st")
                    for g in range(3):
                        nc.vector.bn_stats(stats[:, g],
                                           z[:, g * 256:(g + 1) * 256])
                    mv = mlp_sb.tile([E, 2], F32, tag="mv")
                    nc.vector.bn_aggr(mv, stats)
                    lnv = mlp_sb.tile([E, 1], F32, tag="lnv")
                    nc.scalar.activation(lnv, mv[:, 1:2], AF.Ln,
                                         bias=eps_t[:E])
                    rs = mlp_sb.tile([E, 1], F32, tag="rs")
                    nc.scalar.activation(rs, lnv, AF.Exp, scale=-0.5)
                    nc.vector.tensor_scalar(extLN, z, mv[:, 0:1], rs,
                                            op0=OP.subtract, op1=OP.mult)
                    if use_bias.get("mlp_g"):
                        nc.vector.tensor_mul(extLN, extLN,
                                             bias_t["mlp_g"][:E])
                    if use_bias.get("mlp_b"):
                        nc.vector.tensor_add(extLN, extLN,
                                             bias_t["mlp_b"][:E])
                    if dbg:
                        nc.sync.dma_start(dbg_d["d_extLN"][:],
                                          extLN.bitcast(F32))
                for k in range(KO):
                    _tp(nc, p1_tp, extLN[:, k * P:(k + 1) * P], extLNT[:, k],
                        ident)

            # ---------------- P2: A and wv' ----------------
            with tc.tile_pool(name="p2_sb", bufs=1) as p2_sb, \
                 tc.tile_pool(name="p2_ps", bufs=1, space="PSUM") as p2_ps, \
                 tc.tile_pool(name="p2_tp", bufs=2, space="PSUM") as p2_tp:
                wstarT_sb = p2_sb.tile([128, KO, H], F32R, tag="wstarT")
                nc.sync.dma_start(
                    wstarT_sb, wstarT_d.rearrange("(ko p) n -> p ko n", p=128))
                wvo_sb = p2_sb.tile([128, KO, H], F32R, tag="wvo")
                nc.sync.dma_start(
                    wvo_sb, wvo_d.rearrange("(ko p) n -> p ko n", p=128))
                # A_nat[e, h] = extLN @ WstarT  (== (Wq k_ext.T).T)
                ps = p2_ps.tile([E, H], F32, tag="pa")
                for k in range(KO):
                    for off, ln in ((0, 512), (512, 256)):
                        nc.tensor.matmul(ps[:, off:off + ln], extLNT[:, k],
                                         wstarT_sb[:, k, off:off + ln],
                                         start=(k == 0), stop=(k == KO - 1))
                a_nat = p2_sb.tile([E, H], F32R, tag="a_nat")
                nc.any.tensor_copy(a_nat, ps)
                for k in range(KO):
                    _tp(nc, p2_tp, a_nat[:, k * P:(k + 1) * P], a_t[:, k],
                        ident)
                # wv' = gamma * (extLN @ Wvo) (+ gamma * bv@Wo general term)
                ps2 = p2_ps.tile([E, H], F32, tag="pw")
                for k in range(KO):
                    for off, ln in ((0, 512), (512, 256)):
                        nc.tensor.matmul(ps2[:, off:off + ln], extLNT[:, k],
                                         wvo_sb[:, k, off:off + ln],
                                         start=(k == 0), stop=(k == KO - 1))
                if use_bias.get("bvwo"):
                    nc.vector.tensor_add(ps2, ps2, bias_t["bvwo"][:E])
                nc.vector.tensor_scalar_mul(wvext, ps2, dl_t)
                # cvec[e] = bq . k_ext[e] = extLN[e] . (Wk@bq) + bq.bk
                if use_bias.get("wkbq"):
                    scr = p2_sb.tile([E, H], F32, tag="cscr")
                    cv = p2_sb.tile([E, 1], F32, tag="cv")
                    nc.vector.tensor_mul(scr, extLN.bitcast(F32),
                                         bias_t["wkbq"][:E])
                    nc.vector.reduce_sum(cv, scr, axis=mybir.AxisListType.X)
                    nc.vector.tensor_scalar_add(cv, cv, bias_t["bqbk"][:E])
                    cvr_ps = p2_tp.tile([128, 128], F32, tag="cvp")
                    nc.tensor.transpose(cvr_ps[:1, :E], cv, ident_f[:E, :E])
                    cvr = p2_sb.tile([1, E], F32, tag="cvr")
                    nc.vector.tensor_copy(cvr, cvr_ps[:1, :E])
                    nc.gpsimd.dma_start(cvec_bc, cvr.to_broadcast((128, E)))
                if dbg:
                    nc.sync.dma_start(dbg_d["d_A"][:], a_nat.bitcast(F32))

            # ---------------- M-phase ----------------
            with tc.tile_pool(name="m_tp", bufs=4, space="PSUM") as m_tp:
                for tt in range(TT):
                    for g in range(2):       # two groups of 3 k-tiles
                        ps = m_tp.tile([128, 3, 128], F32R, tag="tp3")
                        for j in range(3):
                            k = g * 3 + j
                            nc.tensor.transpose(
                                ps[:, j], hs_sb[:, tt, k * P:(k + 1) * P],
                                ident)
                        dst = hsT[:, g * 3:(g + 1) * 3,
                                  tt * P:(tt + 1) * P]
                        if (tt * 2 + g) % 2:
                            nc.scalar.copy(dst, ps)
                        else:
                            nc.vector.tensor_copy(dst, ps)

            with tc.tile_pool(name="m_uo", bufs=2, space="PSUM") as m_uo, \
                 tc.tile_pool(name="m_se", bufs=1, space="PSUM") as m_se, \
                 tc.tile_pool(name="m_pgt", bufs=1, space="PSUM") as m_pgt, \
                 tc.tile_pool(name="m_sb", bufs=2) as m_sb, \
                 tc.tile_pool(name="m_sc", bufs=2) as m_sc:
                for tt in range(TT):
                    lhs = [hsT[:, k, tt * P:(tt + 1) * P] for k in range(KO)]
                    uo_ps = m_uo.tile([128, H2], F32, tag="uo")
                    se_ps = m_se.tile([128, E], F32, tag="se")
                    for k in range(KO):
                        for c in range(3):
                            nc.tensor.matmul(
                                uo_ps[:, c * 512:(c + 1) * 512], lhs[k],
                                wcat_sb[:, k, c * 512:(c + 1) * 512],
                                start=(k == 0), stop=(k == KO - 1))
                        nc.tensor.matmul(se_ps, lhs[k], a_t[:, k],
                                         start=(k == 0), stop=(k == KO - 1))
                    u_ps = uo_ps[:, 0:H]
                    ov_ps = uo_ps[:, H:H2]
                    hs_f = hs_sb[:, tt].bitcast(F32)
                    if use_bias.get("dvec"):
                        nc.vector.tensor_add(u_ps, u_ps, bias_t["dvec"])
                    scr = m_sc.tile([128, H], F32, tag="scr")
                    ss = m_sc.tile([128, 1], F32, tag="ss")
                    nc.vector.tensor_mul(scr, u_ps, hs_f)
                    nc.vector.reduce_sum(ss, scr, axis=mybir.AxisListType.X)
                    if use_bias.get("c0"):
                        nc.vector.tensor_scalar_add(ss, ss, bias_t["c0"])
                    e0 = m_sc.tile([128, 1], F32, tag="e0")
                    nc.scalar.activation(e0, ss, AF.Exp)
                    if use_bias.get("wkbq"):
                        nc.vector.tensor_add(se_ps, se_ps, cvec_bc)
                    eext = m_sc.tile([128, E], F32, tag="eext")
                    zext = m_sc.tile([128, 1], F32, tag="zext")
                    nc.scalar.activation(eext, se_ps, AF.Exp, accum_out=zext)
                    if dbg:
                        se_cp = m_sc.tile([128, E], F32, tag="se_cp")
                        nc.vector.tensor_copy(se_cp, se_ps)
                        nc.sync.dma_start(
                            dbg_d["d_sext"]
                            [:].rearrange("(tt p) e -> p tt e", p=128)[:, tt],
                            se_cp)
                        nc.sync.dma_start(dbg_d["d_ss"][:, tt:tt + 1], ss)
                    z_t = m_sc.tile([128, 1], F32, tag="z")
                    nc.vector.tensor_add(z_t, zext, e0)
                    rz = m_sc.tile([128, 1], F32, tag="rz")
                    nc.vector.reciprocal(rz, z_t)
                    p0 = m_sc.tile([128, 1], F32, tag="p0")
                    nc.vector.tensor_mul(p0, e0, rz)
                    pg = m_sc.tile([128, E], F32R, tag="pg")
                    nc.vector.tensor_scalar_mul(pg, eext, rz)
                    pgT_ps = m_pgt.tile([E, 128], F32R, tag="pgT")
                    nc.tensor.transpose(pgT_ps, pg, ident)
                    pgT = m_sc.tile([E, 128], F32R, tag="pgTs")
                    nc.vector.tensor_copy(pgT, pgT_ps)
                    # sb1 = p0 * ov (ACT Copy+scale; Copy is in every
                    # act table set so this forces no table reload)
                    sb1 = m_sb.tile([128, H], F32, tag="sb1")
                    nc.scalar.activation(sb1, ov_ps, AF.Copy, scale=p0)
                    for off, ln in ((0, 256), (256, 512)):
                        nc.tensor.matmul(ov_ps[:, off:off + ln], pgT,
                                         wvext[:, off:off + ln],
                                         start=True, stop=True)
                    sbz = m_sb.tile([128, H], F32, tag="sbz")
                    nc.vector.tensor_add(sbz, sb1, ov_ps)
                    if use_bias.get("bo"):
                        nc.vector.tensor_add(sbz, sbz, bias_t["bo"])
                    nc.gpsimd.tensor_add(sbz, sbz, hs_f)
                    # LayerNorm over H; rstd = Exp(-0.5 * Ln(var + eps))
                    stats = m_sc.tile([128, 3, 6], F32, tag="lnst")
                    for g in range(3):
                        nc.vector.bn_stats(stats[:, g],
                                           sbz[:, g * 256:(g + 1) * 256])
                    mv = m_sc.tile([128, 2], F32, tag="lnmv")
                    nc.vector.bn_aggr(mv, stats)
                    lnv = m_sc.tile([128, 1], F32, tag="lnv")
                    nc.scalar.activation(lnv, mv[:, 1:2], AF.Ln, bias=eps_t)
                    rs = m_sc.tile([128, 1], F32, tag="lnrs")
                    nc.scalar.activation(rs, lnv, AF.Exp, scale=-0.5)
                    fin = m_sb.tile([128, H], F32, tag="fin")
                    nc.vector.tensor_scalar(fin, sbz, mv[:, 0:1], rs,
                                            op0=OP.subtract, op1=OP.mult)
                    if use_bias.get("ln_g"):
                        nc.vector.tensor_mul(fin, fin, bias_t["ln_g"])
                    if use_bias.get("ln_b"):
                        nc.vector.tensor_add(fin, fin, bias_t["ln_b"])
                    nc.sync.dma_start(
                        out_d[:].rearrange("(tt p) h -> p tt h", p=128)[:, tt],
                        fin)

    nc.finalize()
    return nc


_CACHE = {}


def _get_nc(use_bias, dbg=False):
    key = (tuple(sorted(use_bias.items())), dbg)
    if key not in _CACHE:
        _CACHE[key] = _build(use_bias, dbg)
    return _CACHE[key]


def _fold_weights(w):
    """Host-side fp64 constant folds of weight-only products."""
    wq = w["Wq"].astype(np.float64)
    wk = w["Wk"].astype(np.float64)
    wv = w["Wv"].astype(np.float64)
    wo = w["Wo"].astype(np.float64)
    wstar = wq @ wk.T
    wvo = wv @ wo
    return {
        "Wcat": np.ascontiguousarray(
            np.concatenate([wstar, wvo], axis=1), dtype=np.float32),
        "WstarT": np.ascontiguousarray(wstar.T, dtype=np.float32),
        "Wvo": np.ascontiguousarray(wvo, dtype=np.float32),
    }


def _use_bias_flags(w):
    any_qk = bool(np.any(w["bq"])) or bool(np.any(w["bk"]))
    return {
        "b1": bool(np.any(w["b1"])), "b2": bool(np.any(w["b2"])),
        "bo": bool(np.any(w["bo"])),
        "bvwo": bool(np.any(w["bv"])),
        "mlp_g": bool(np.any(w["mlp_ln_g"] != 1.0)),
        "mlp_b": bool(np.any(w["mlp_ln_b"])),
        "ln_g": bool(np.any(w["ln_g"] != 1.0)),
        "ln_b": bool(np.any(w["ln_b"])),
        "dvec": any_qk, "c0": any_qk,
        "wkbq": bool(np.any(w["bq"])), "bqbk": bool(np.any(w["bq"])),
    }


def _prep(inputs):
    """Returns (use_bias, in_maps)."""
    hs = np.ascontiguousarray(inputs["hidden_states"], dtype=np.float32)
    ext = np.ascontiguousarray(inputs["external_embeddings"], dtype=np.float32)
    dl = np.ascontiguousarray(inputs["doc_logprobs"], dtype=np.float32)
    names = ["Wq", "bq", "Wk", "bk", "Wv", "bv", "Wo", "bo", "ln_g", "ln_b",
             "W1", "b1", "W2", "b2", "mlp_ln_g", "mlp_ln_b"]
    w = {n: np.ascontiguousarray(inputs[n], dtype=np.float32) for n in names}
    use_bias = _use_bias_flags(w)
    base = _fold_weights(w)
    base["W1"] = w["W1"]
    base["W2"] = w["W2"]
    for nm, src in (("b1", "b1"), ("b2", "b2"), ("bo", "bo"),
                    ("mlp_g", "mlp_ln_g"), ("mlp_b", "mlp_ln_b"),
                    ("ln_g", "ln_g"), ("ln_b", "ln_b")):
        if use_bias[nm]:
            base[nm] = w[src].reshape(1, -1)
    if use_bias["bvwo"]:
        base["bvwo"] = (w["bv"].astype(np.float64)
                        @ w["Wo"].astype(np.float64)
                        ).astype(np.float32).reshape(1, H)
    if use_bias["dvec"]:
        base["dvec"] = (w["Wq"].astype(np.float64) @ w["bk"]
                        + w["Wk"].astype(np.float64) @ w["bq"]
                        ).astype(np.float32).reshape(1, H)
        base["c0"] = np.dot(w["bq"], w["bk"]).reshape(1, 1).astype(np.float32)
    if use_bias["wkbq"]:
        base["wkbq"] = (w["Wk"].astype(np.float64) @ w["bq"]
                        ).astype(np.float32).reshape(1, H)
        base["bqbk"] = np.dot(w["bq"], w["bk"]).reshape(1, 1).astype(
            np.float32)
    in_maps = []
    for c in range(B):
        m = dict(base)
        m["hs"] = hs[c]
        m["ext"] = ext[c]
        m["dl"] = dl[c].reshape(E, 1)
        in_maps.append(m)
    return use_bias, in_maps


def kernel(**inputs) -> np.ndarray:
    use_bias, in_maps = _prep(inputs)
    nc = _get_nc(use_bias)
    res = run_bass_kernel_spmd(nc, in_maps, core_ids=list(range(B)))
    return np.stack([res.results[c]["out"] for c in range(B)], axis=0)


def timed_run(inputs):
    """Run with tracing on all cores; returns max per-core exec time in ns."""
    use_bias, in_maps = _prep(inputs)
    nc = _get_nc(use_bias)
    res = run_bass_kernel_spmd(nc, in_maps, core_ids=list(range(B)),
                               trace=True, trace_cores=list(range(B)),
                               stitch_traces=False)
    if res.exec_time_ns is None:
        raise RuntimeError("no exec time in results (trace hook missing?)")
    print(f"per-core mean exec: {res.mean_exec_time_ns} ns, "
          f"max core: {res.max_exec_time_core_id}")
    if res.instructions_and_trace is not None:
        print(f"trace: {res.instructions_and_trace[1]}")
    return res.exec_time_ns



# revision 20
# speedup vs baseline: 1.3663x; 1.3663x over previous
"""Trainium2 Bass kernel for nn_ExternalEmbeddingAttention.

Sharding: data-parallel over batch B=8 across 8 NeuronCores (one example per
core); weights replicated.

Host constant-folds (weight-only algebra, computed once in fp64 in kernel()):
  W*   = Wq @ Wk.T        W*T = Wk @ Wq.T        Wvo = Wv @ Wo
  Wcat = [W* | Wvo]   ([H, 2H], shares one stationary per token/k tile)
All large tensors travel and compute in bf16 (fp32 PSUM accumulation); the
per-element error stays ~0.4% which is far inside the 2e-2 scale-relative
gate.  PE cost model: a matmul streams its MOVING free dim at 1 col/cycle, so
the layout is chosen to keep the moving operand wide only where the math
needs it (hs @ Wcat) and N=16 everywhere else:
  ext MLP:  h1T[i,e]  = W1-ktile stationary  x extT moving   (N=16)
            h2T[h,e]  = W2-jtile stationary  x gelu(h1T)     (N=16)
  a_t[h,e]  = WstarT-tile stationary x extLNT moving          (N=16)
  se[s,e]   = hsT-tile stationary x a_t moving                (N=16)
Token phase is split: stage A (per tile: u = hs@W*, ov = hs@Wvo, s_self via a
fused DVE multiply-reduce, ov drained to SBUF) runs as one dense PE stream so
the HAM clock gate stays at 2.4 GHz; stage B (softmax, P@wv', residual+LN)
is interleaved 2-per-A once a_t/wv' exist, so DVE/ACT work hides under the
PE-bound stage-A windows.
"""

import numpy as np
import ml_dtypes

import concourse.bass as bass
import concourse.tile as tile
import concourse.mybir as mybir
from concourse import bacc
from concourse.bass_utils import run_bass_kernel_spmd
from concourse.masks import make_identity
import concourse.bass_utils as _bass_utils

# Walrus's LDWEIGHTS-dedup pass is disabled by default in this harness; with
# fused matmuls every MATMUL re-loads its stationary operand otherwise.
_orig_run_command = _bass_utils.run_command


_LDW_OPT = False  # bf16 transpose LDWEIGHTS breaks walrus's ldw-opt pass


def _run_command_ldwopt(argv, **kwargs):
    if _LDW_OPT:
        argv = ["--enable-ldw-opt=true" if a == "--enable-ldw-opt=false"
                else a for a in argv]
    return _orig_run_command(argv, **kwargs)


if _bass_utils.run_command is not _run_command_ldwopt:
    _bass_utils.run_command = _run_command_ldwopt

# Steer the act-table chooser: Exp and Ln both live in
# natural_log_exp_and_others; restrict them to that shared set so the
# scheduler never reloads activation tables mid-loop.
from concourse.hw_specs import get_activation_tables as _gat


def _steer_act_tables(arch="gen3"):
    t = _gat(arch)   # functools.cache -> in-place mutation persists
    for name, funcs in t.items():
        if name != "natural_log_exp_and_others":
            funcs.discard(mybir.ActivationFunctionType.Exp)
            funcs.discard(mybir.ActivationFunctionType.Ln)


_steer_act_tables()

F32 = mybir.dt.float32
BF = mybir.dt.bfloat16
AF = mybir.ActivationFunctionType
OP = mybir.AluOpType

B, S, H, E, I = 8, 2048, 768, 16, 3072
EPS = 1e-12
P = 128
KO = H // P          # 6 k-tiles over a 768 dim
TT = S // P          # 16 token tiles
JO = I // P          # 24 i-tiles over 3072
H2 = 2 * H
NPBF = np.dtype(ml_dtypes.bfloat16)


_STAGE = "full"   # debug bisect knob: dma | A | mlp | se | full


def _build(use_bias: dict):
    nc = bacc.Bacc()

    hs_d = nc.dram_tensor("hs", [S, H], BF, kind="ExternalInput")
    ext_d = nc.dram_tensor("ext", [E, H], BF, kind="ExternalInput")
    dl_d = nc.dram_tensor("dl", [E, 1], F32, kind="ExternalInput")
    wcat_d = nc.dram_tensor("Wcat", [H, H2], BF, kind="ExternalInput")
    wstarT_d = nc.dram_tensor("WstarT", [H, H], BF, kind="ExternalInput")
    w1_d = nc.dram_tensor("W1", [H, I], BF, kind="ExternalInput")
    w2_d = nc.dram_tensor("W2", [I, H], BF, kind="ExternalInput")
    bias_d = {}
    for nm, sz in (("b1", I), ("b2", H), ("bo", H), ("mlp_g", H),
                   ("mlp_b", H), ("ln_g", H), ("ln_b", H), ("dvec", H),
                   ("c0", 1), ("wkbq", H), ("bqbk", 1), ("bvwo", H)):
        if use_bias.get(nm):
            bias_d[nm] = nc.dram_tensor(nm, [1, sz], F32, kind="ExternalInput")
    out_d = nc.dram_tensor("out", [S, H], BF, kind="ExternalOutput")

    hs_r = hs_d.rearrange("(tt p) h -> p tt h", p=128)
    w1_r = w1_d.rearrange("(ko p) n -> p ko n", p=128)
    w2_r = w2_d.rearrange("(jo p) n -> p jo n", p=128)
    out_r = out_d.rearrange("(tt p) h -> p tt h", p=128)

    with tile.TileContext(nc) as tc:
        with tc.tile_pool(name="persist", bufs=1) as persist, \
             tc.tile_pool(name="work", bufs=2) as work, \
             tc.tile_pool(name="wpool", bufs=1) as wpool, \
             tc.tile_pool(name="uo_ps", bufs=2, space="PSUM") as uo_pool:

            # ---------------- persistent tiles ----------------
            ident_f = persist.tile([128, 128], F32, tag="ident_f")
            make_identity(nc, ident_f)
            ident = persist.tile([128, 128], BF, tag="ident")
            nc.vector.tensor_copy(ident, ident_f)
            eps_t = persist.tile([128, 1], F32, tag="eps")
            nc.vector.memset(eps_t, EPS)
            dl_t = persist.tile([E, 1], F32, tag="dl")
            nc.sync.dma_start(dl_t, dl_d[:])

            bias_t = {}
            for nm, d in bias_d.items():
                sz = d.shape[1]
                pp = E if nm in ("b1", "b2", "mlp_g", "mlp_b", "wkbq") else P
                t = persist.tile([pp, sz], F32, tag=f"bias_{nm}",
                                 name=f"bias_{nm}")
                nc.gpsimd.dma_start(t, d[:].to_broadcast((pp, sz)))
                bias_t[nm] = t

            hs_sb = persist.tile([128, TT, H], BF, tag="hs")
            hsT = persist.tile([128, KO, S], BF, tag="hsT")
            wcat_sb = persist.tile([128, KO, H2], BF, tag="wcat")
            wstarT_sb = persist.tile([128, KO, H], BF, tag="wstarT")
            ov_sb = persist.tile([128, TT, H], BF, tag="ov_sb")
            ss_all = persist.tile([128, TT], F32, tag="ss_all")
            se_sb = persist.tile([128, TT, E], F32, tag="se_sb")
            ext_t = persist.tile([E, H], BF, tag="ext_t")
            extT = persist.tile([128, KO, E], BF, tag="extT")
            extLN = persist.tile([E, H], BF, tag="extLN")
            extLNT = persist.tile([128, KO, E], BF, tag="extLNT")
            a_t = persist.tile([128, KO, E], BF, tag="a_t")
            wvext = persist.tile([E, H], BF, tag="wvext")
            cvec_bc = (persist.tile([128, E], F32, tag="cvec_bc")
                       if use_bias.get("wkbq") else None)

            # ---------------- DMA issue (sync ring, priority order) -------
            nc.sync.dma_start(ext_t, ext_d[:])
            for tt in range(8):
                nc.sync.dma_start(hs_sb[:, tt], hs_r[:, tt])
            nc.sync.dma_start(wcat_sb,
                              wcat_d.rearrange("(ko p) n -> p ko n", p=128))
            w1_sb = wpool.tile([128, KO, I], BF, tag="w1")
            w2_sb = wpool.tile([128, JO, H], BF, tag="w2")
            for c in range(2):
                nc.sync.dma_start(w1_sb[:, :, c * 1536:(c + 1) * 1536],
                                  w1_r[:, :, c * 1536:(c + 1) * 1536])
            nc.sync.dma_start(wstarT_sb,
                              wstarT_d.rearrange("(ko p) n -> p ko n", p=128))
            for tt in range(8, 12):
                nc.sync.dma_start(hs_sb[:, tt], hs_r[:, tt])
            nc.sync.dma_start(w2_sb[:, 0:12], w2_r[:, 0:12])
            for tt in range(12, TT):
                nc.sync.dma_start(hs_sb[:, tt], hs_r[:, tt])
            nc.sync.dma_start(w2_sb[:, 12:24], w2_r[:, 12:24])

            # ---------------- emission helpers ----------------
            def tp128(pool, dst_ap, src_ap, eng):
                """PE transpose [p,f]->[f,p] via PSUM, copy out on eng."""
                pdim = src_ap.shape[-1]
                fdim = src_ap.shape[0]
                ps = pool.tile([128, 128], BF, tag="tp")
                nc.tensor.transpose(ps[:pdim, :fdim], src_ap,
                                    ident[:fdim, :fdim])
                if eng == "act":
                    nc.scalar.copy(dst_ap, ps[:pdim, :fdim])
                else:
                    nc.vector.tensor_copy(dst_ap, ps[:pdim, :fdim])

            def emit_hsT(tt, pool):
                for k in range(KO):
                    tp128(pool, hsT[:, k, tt * P:(tt + 1) * P],
                          hs_sb[:, tt, k * P:(k + 1) * P],
                          "act" if (tt * KO + k) % 2 else "dve")

            def emit_A(tt):
                """u/ov matmuls + s_self + ov drain for one token tile."""
                uo = uo_pool.tile([128, H2], F32, tag="uo")
                for k in range(KO):
                    lhs = hsT[:, k, tt * P:(tt + 1) * P]
                    for c in range(3):
                        nc.tensor.matmul(
                            uo[:, c * 512:(c + 1) * 512], lhs,
                            wcat_sb[:, k, c * 512:(c + 1) * 512],
                            start=(k == 0), stop=(k == KO - 1))
                u_ps = uo[:, 0:H]
                ov_ps = uo[:, H:H2]
                if use_bias.get("dvec"):
                    nc.vector.tensor_add(u_ps, u_ps, bias_t["dvec"])
                scr = work.tile([128, H], BF, tag="scr")
                nc.vector.tensor_mul(scr, u_ps, hs_sb[:, tt])
                nc.vector.reduce_sum(ss_all[:, tt:tt + 1], scr,
                                     axis=mybir.AxisListType.X)
                # drain on DVE: keeps every PSUM consumer of this tile on one
                # engine, so the later Pv overwrite can never collide with a
                # still-pending ACT read of the shared middle bank
                nc.vector.tensor_copy(ov_sb[:, tt], ov_ps)

            def emit_mlp_w1(mp):
                h1_ps = mp.tile([128, JO, E], F32, tag="h1T")
                for j in range(JO):
                    for k in range(KO):
                        nc.tensor.matmul(
                            h1_ps[:, j], w1_sb[:, k, j * P:(j + 1) * P],
                            extT[:, k], start=(k == 0), stop=(k == KO - 1))
                h1g = wpool.tile([128, JO, E], BF, tag="h1g")
                if use_bias.get("b1"):
                    b1v = wpool.tile([128, JO, 1], F32, tag="b1v")
                    nc.sync.dma_start(
                        b1v, bias_d["b1"][:].rearrange(
                            "o (jo p) -> p jo o", p=128))
                    for j in range(JO):
                        nc.scalar.activation(h1g[:, j], h1_ps[:, j], AF.Gelu,
                                             bias=b1v[:, j])
                else:
                    nc.scalar.activation(h1g, h1_ps, AF.Gelu)
                return h1g

            def emit_mlp_w2(h1g, mp):
                # one accumulation group at a time: start=True zeroes the
                # has_written bits of the WHOLE bank, so groups in one bank
                # must not interleave
                h2_ps = mp.tile([128, KO, E], F32, tag="h2T")
                for k in range(KO):
                    for j in range(JO):
                        nc.tensor.matmul(
                            h2_ps[:, k], w2_sb[:, j, k * P:(k + 1) * P],
                            h1g[:, j], start=(j == 0), stop=(j == JO - 1))
                h2T_sb = wpool.tile([128, KO, E], BF, tag="h2T_sb")
                nc.vector.tensor_copy(h2T_sb, h2_ps)
                z = wpool.tile([E, H], F32, tag="z")
                for k in range(KO):
                    tp128(mp, z[:, k * P:(k + 1) * P], h2T_sb[:, k],
                          "act" if k % 2 else "dve")
                # residual + LN over free dim (16 partitions)
                nc.vector.tensor_add(z, z, ext_t)
                if use_bias.get("b2"):
                    nc.vector.tensor_add(z, z, bias_t["b2"][:E])
                stats = wpool.tile([E, 3, 6], F32, tag="st")
                for g in range(3):
                    nc.vector.bn_stats(stats[:, g],
                                       z[:, g * 256:(g + 1) * 256])
                mv = wpool.tile([E, 2], F32, tag="mv")
                nc.vector.bn_aggr(mv, stats)
                lnv = wpool.tile([E, 1], F32, tag="lnv")
                nc.scalar.activation(lnv, mv[:, 1:2], AF.Ln, bias=eps_t[:E])
                rs = wpool.tile([E, 1], F32, tag="rs")
                nc.scalar.activation(rs, lnv, AF.Exp, scale=-0.5)
                nc.vector.tensor_scalar(extLN, z, mv[:, 0:1], rs,
                                        op0=OP.subtract, op1=OP.mult)
                if use_bias.get("mlp_g"):
                    nc.vector.tensor_mul(extLN, extLN, bias_t["mlp_g"][:E])
                if use_bias.get("mlp_b"):
                    nc.vector.tensor_add(extLN, extLN, bias_t["mlp_b"][:E])

            def emit_p2(mp):
                for k in range(KO):
                    tp128(mp, extLNT[:, k], extLN[:, k * P:(k + 1) * P],
                          "act" if k % 2 else "dve")
                # a_t[:, k][h, e] = sum_h' W*[kh, h'] extLN[e, h']
                at_ps = mp.tile([128, KO, E], F32, tag="at")
                for k in range(KO):
                    for kp in range(KO):
                        nc.tensor.matmul(
                            at_ps[:, k],
                            wstarT_sb[:, kp, k * P:(k + 1) * P],
                            extLNT[:, kp], start=(kp == 0),
                            stop=(kp == KO - 1))
                nc.vector.tensor_copy(a_t, at_ps)
                # cvec[e] = bq . k_ext[e]  (general-bias path)
                if use_bias.get("wkbq"):
                    scrq = wpool.tile([E, H], F32, tag="cscr")
                    cv = wpool.tile([E, 1], F32, tag="cv")
                    nc.vector.tensor_mul(scrq, extLN, bias_t["wkbq"][:E])
                    nc.vector.reduce_sum(cv, scrq, axis=mybir.AxisListType.X)
                    nc.vector.tensor_scalar_add(cv, cv, bias_t["bqbk"][:E])
                    cvp = mp.tile([128, 128], F32, tag="cvp")
                    nc.tensor.transpose(cvp[:1, :E], cv, ident_f[:E, :E])
                    cvr = wpool.tile([1, E], F32, tag="cvr")
                    nc.vector.tensor_copy(cvr, cvp[:1, :E])
                    nc.gpsimd.dma_start(cvec_bc, cvr.to_broadcast((128, E)))

            def emit_wv(mp):
                # wv' = gamma * (extLN @ Wvo) reusing the Wvo half of wcat
                wv_ps = mp.tile([E, 384], F32, tag="wv")
                for hf in range(2):
                    for k in range(KO):
                        nc.tensor.matmul(
                            wv_ps, extLNT[:, k],
                            wcat_sb[:, k, H + hf * 384:H + (hf + 1) * 384],
                            start=(k == 0), stop=(k == KO - 1))
                    if use_bias.get("bvwo"):
                        nc.vector.tensor_add(
                            wv_ps, wv_ps,
                            bias_t["bvwo"][:E, hf * 384:(hf + 1) * 384])
                    nc.vector.tensor_scalar_mul(
                        wvext[:, hf * 384:(hf + 1) * 384], wv_ps, dl_t)

            def emit_se_all(mp):
                se_ps = mp.tile([128, E], F32, tag="se")
                for tt in range(TT):
                    for k in range(KO):
                        nc.tensor.matmul(se_ps,
                                         hsT[:, k, tt * P:(tt + 1) * P],
                                         a_t[:, k], start=(k == 0),
                                         stop=(k == KO - 1))
                    if use_bias.get("wkbq"):
                        nc.vector.tensor_add(se_sb[:, tt], se_ps, cvec_bc)
                    else:
                        nc.vector.tensor_copy(se_sb[:, tt], se_ps)

            def emit_B(tt, pgt_pool):
                """softmax + P@wv' + output dense tail for one token tile."""
                eext = work.tile([128, E], F32, tag="eext")
                zext = work.tile([128, 1], F32, tag="zext")
                nc.scalar.activation(eext, se_sb[:, tt], AF.Exp,
                                     accum_out=zext)
                e0 = work.tile([128, 1], F32, tag="e0")
                if use_bias.get("c0"):
                    nc.scalar.activation(e0, ss_all[:, tt:tt + 1], AF.Exp,
                                         bias=bias_t["c0"])
                else:
                    nc.scalar.activation(e0, ss_all[:, tt:tt + 1], AF.Exp)
                z_t = work.tile([128, 1], F32, tag="z")
                nc.vector.tensor_add(z_t, zext, e0)
                rz = work.tile([128, 1], F32, tag="rz")
                nc.vector.reciprocal(rz, z_t)
                p0 = work.tile([128, 1], F32, tag="p0")
                nc.vector.tensor_mul(p0, e0, rz)
                pg = work.tile([128, E], BF, tag="pg")
                nc.vector.tensor_scalar_mul(pg, eext, rz)
                pgT_ps = pgt_pool.tile([E, 128], BF, tag="pgT")
                nc.tensor.transpose(pgT_ps, pg, ident)
                pgT = work.tile([E, 128], BF, tag="pgTs")
                nc.vector.tensor_copy(pgT, pgT_ps)
                # sb1 = p0 * ov   (ACT Copy+scale, from SBUF)
                sb1 = work.tile([128, H], BF, tag="sb1")
                nc.scalar.activation(sb1, ov_sb[:, tt], AF.Copy, scale=p0)
                # Pv overwrites the ov half of a retired uo buffer: full
                # address overlap with that tile's DVE drain orders it safely
                pv = uo_pool.tile([128, H2], F32, tag="uo")
                for off, ln in ((H, 256), (H + 256, 512)):
                    nc.tensor.matmul(pv[:, off:off + ln], pgT,
                                     wvext[:, off - H:off - H + ln],
                                     start=True, stop=True)
                out2 = work.tile([128, H], BF, tag="out2")
                nc.vector.tensor_add(out2, sb1, pv[:, H:H2])
                if use_bias.get("bo"):
                    nc.vector.tensor_add(out2, out2, bias_t["bo"])
                sbz = work.tile([128, H], BF, tag="sbz")
                nc.gpsimd.tensor_add(sbz, out2, hs_sb[:, tt])
                # LayerNorm over H; rstd = Exp(-0.5 * Ln(var + eps))
                stats = work.tile([128, 3, 6], F32, tag="lnst")
                for g in range(3):
                    nc.vector.bn_stats(stats[:, g],
                                       sbz[:, g * 256:(g + 1) * 256])
                mv = work.tile([128, 2], F32, tag="lnmv")
                nc.vector.bn_aggr(mv, stats)
                lnv = work.tile([128, 1], F32, tag="lnv")
                nc.scalar.activation(lnv, mv[:, 1:2], AF.Ln, bias=eps_t)
                rs = work.tile([128, 1], F32, tag="lnrs")
                nc.scalar.activation(rs, lnv, AF.Exp, scale=-0.5)
                fin = work.tile([128, H], BF, tag="fin")
                nc.vector.tensor_scalar(fin, sbz, mv[:, 0:1], rs,
                                        op0=OP.subtract, op1=OP.mult)
                if use_bias.get("ln_g"):
                    nc.vector.tensor_mul(fin, fin, bias_t["ln_g"])
                if use_bias.get("ln_b"):
                    nc.vector.tensor_add(fin, fin, bias_t["ln_b"])
                nc.scalar.dma_start(out_r[:, tt], fin)

            # ---------------- schedule ----------------
            stg = {"dma": 0, "A": 1, "mlp": 2, "se": 3, "full": 4}[_STAGE]
            with tc.tile_pool(name="tpA", bufs=2, space="PSUM") as tpa:
                for k in range(KO):
                    tp128(tpa, extT[:, k], ext_t[:, k * P:(k + 1) * P],
                          "act" if k % 2 else "dve")
                emit_hsT(0, tpa)
                emit_hsT(1, tpa)
                if stg >= 1:
                    emit_A(0)
                emit_hsT(2, tpa)
                emit_hsT(3, tpa)
                if stg >= 1:
                    emit_A(1)
                emit_hsT(4, tpa)
                emit_hsT(5, tpa)
                if stg >= 1:
                    emit_A(2)
                emit_hsT(6, tpa)
                emit_hsT(7, tpa)
                if stg >= 1:
                    emit_A(3)
                    emit_A(4)
            if stg >= 2:
                with tc.tile_pool(name="mlp1", bufs=1, space="PSUM") as mp:
                    h1g = emit_mlp_w1(mp)
            if stg >= 1:
                emit_A(5)
                emit_A(6)
            if stg >= 2:
                with tc.tile_pool(name="mlp2", bufs=1, space="PSUM") as mp:
                    emit_mlp_w2(h1g, mp)
            with tc.tile_pool(name="tpB", bufs=2, space="PSUM") as tpb:
                for tt in range(8, TT):
                    emit_hsT(tt, tpb)
            if stg >= 2:
                with tc.tile_pool(name="p2a", bufs=1, space="PSUM") as mp:
                    emit_p2(mp)
                with tc.tile_pool(name="p2b", bufs=1, space="PSUM") as mp:
                    emit_wv(mp)
            if stg >= 3:
                with tc.tile_pool(name="sep", bufs=1, space="PSUM") as mp:
                    emit_se_all(mp)
            if stg >= 4:
                with tc.tile_pool(name="pgt_ps", bufs=1,
                                  space="PSUM") as pgt_pool:
                    bq = list(range(TT))   # B backlog
                    for tt in range(7, TT):
                        emit_A(tt)
                        for _ in range(2):
                            if bq and bq[0] < tt:
                                emit_B(bq.pop(0), pgt_pool)
                    while bq:
                        emit_B(bq.pop(0), pgt_pool)
            else:
                if stg >= 1:
                    for tt in range(7, TT):
                        emit_A(tt)
                for tt in range(TT):
                    src = ov_sb[:, tt] if stg >= 1 else hs_sb[:, tt]
                    nc.scalar.dma_start(out_r[:, tt], src)

    nc.finalize()
    return nc


_CACHE = {}


def _get_nc(use_bias):
    key = tuple(sorted(use_bias.items()))
    if key not in _CACHE:
        _CACHE[key] = _build(use_bias)
    return _CACHE[key]


def _fold_weights(w):
    """Host-side fp64 constant folds of weight-only products."""
    wq = w["Wq"].astype(np.float64)
    wk = w["Wk"].astype(np.float64)
    wv = w["Wv"].astype(np.float64)
    wo = w["Wo"].astype(np.float64)
    wstar = wq @ wk.T
    wvo = wv @ wo
    return {
        "Wcat": np.ascontiguousarray(
            np.concatenate([wstar, wvo], axis=1)).astype(NPBF),
        "WstarT": np.ascontiguousarray(wstar.T).astype(NPBF),
    }


def _use_bias_flags(w):
    any_qk = bool(np.any(w["bq"])) or bool(np.any(w["bk"]))
    return {
        "b1": bool(np.any(w["b1"])), "b2": bool(np.any(w["b2"])),
        "bo": bool(np.any(w["bo"])),
        "bvwo": bool(np.any(w["bv"])),
        "mlp_g": bool(np.any(w["mlp_ln_g"] != 1.0)),
        "mlp_b": bool(np.any(w["mlp_ln_b"])),
        "ln_g": bool(np.any(w["ln_g"] != 1.0)),
        "ln_b": bool(np.any(w["ln_b"])),
        "dvec": any_qk, "c0": any_qk,
        "wkbq": bool(np.any(w["bq"])), "bqbk": bool(np.any(w["bq"])),
    }


def _prep(inputs):
    """Returns (use_bias, in_maps)."""
    hs = np.ascontiguousarray(inputs["hidden_states"],
                              dtype=np.float32).astype(NPBF)
    ext = np.ascontiguousarray(inputs["external_embeddings"],
                               dtype=np.float32).astype(NPBF)
    dl = np.ascontiguousarray(inputs["doc_logprobs"], dtype=np.float32)
    names = ["Wq", "bq", "Wk", "bk", "Wv", "bv", "Wo", "bo", "ln_g", "ln_b",
             "W1", "b1", "W2", "b2", "mlp_ln_g", "mlp_ln_b"]
    w = {n: np.ascontiguousarray(inputs[n], dtype=np.float32) for n in names}
    use_bias = _use_bias_flags(w)
    base = _fold_weights(w)
    base["W1"] = w["W1"].astype(NPBF)
    base["W2"] = w["W2"].astype(NPBF)
    for nm, src in (("b1", "b1"), ("b2", "b2"), ("bo", "bo"),
                    ("mlp_g", "mlp_ln_g"), ("mlp_b", "mlp_ln_b"),
                    ("ln_g", "ln_g"), ("ln_b", "ln_b")):
        if use_bias[nm]:
            base[nm] = w[src].reshape(1, -1)
    if use_bias["bvwo"]:
        base["bvwo"] = (w["bv"].astype(np.float64)
                        @ w["Wo"].astype(np.float64)
                        ).astype(np.float32).reshape(1, H)
    if use_bias["dvec"]:
        base["dvec"] = (w["Wq"].astype(np.float64) @ w["bk"]
                        + w["Wk"].astype(np.float64) @ w["bq"]
                        ).astype(np.float32).reshape(1, H)
        base["c0"] = np.dot(w["bq"], w["bk"]).reshape(1, 1).astype(np.float32)
    if use_bias["wkbq"]:
        base["wkbq"] = (w["Wk"].astype(np.float64) @ w["bq"]
                        ).astype(np.float32).reshape(1, H)
        base["bqbk"] = np.dot(w["bq"], w["bk"]).reshape(1, 1).astype(
            np.float32)
    in_maps = []
    for c in range(B):
        m = dict(base)
        m["hs"] = hs[c]
        m["ext"] = ext[c]
        m["dl"] = dl[c].reshape(E, 1)
        in_maps.append(m)
    return use_bias, in_maps


def kernel(**inputs) -> np.ndarray:
    use_bias, in_maps = _prep(inputs)
    nc = _get_nc(use_bias)
    res = run_bass_kernel_spmd(nc, in_maps, core_ids=list(range(B)))
    return np.stack([res.results[c]["out"].astype(np.float32)
                     for c in range(B)], axis=0)


def timed_run(inputs):
    """Run with tracing on all cores; returns max per-core exec time in ns."""
    use_bias, in_maps = _prep(inputs)
    nc = _get_nc(use_bias)
    res = run_bass_kernel_spmd(nc, in_maps, core_ids=list(range(B)),
                               trace=True, trace_cores=list(range(B)),
                               stitch_traces=False)
    if res.exec_time_ns is None:
        raise RuntimeError("no exec time in results (trace hook missing?)")
    print(f"per-core mean exec: {res.mean_exec_time_ns} ns, "
          f"max core: {res.max_exec_time_core_id}")
    if res.instructions_and_trace is not None:
        print(f"trace: {res.instructions_and_trace[1]}")
    return res.exec_time_ns
